# revision 15
# baseline (speedup 1.0000x reference)
"""Distributed causal multi-head attention for Trainium2 (8 NeuronCores).

Reference computes, for x [2, 2048, 1024]:
    qkv = x @ W_qkv + b_qkv ; split into q,k,v heads (16 heads, d_k=64)
    causal softmax attention per head
    out = ctx @ W_o + b_o

Sharding (data + head parallel): core c handles batch b=c//4 and heads
H = [4g..4g+3] with g=c%4.  Per core:
  - QKV projections into q^T,k^T ([dk, T], head-pairs packed into 128
    partitions) and v ([T, dk] + ones column so the attention-weights
    matmul also produces softmax denominators),
  - causal attention processed q-chunk (TQ=512) outer, head-pairs inner;
    logits for the two heads of a pair land in one [128, 2048] PSUM tile
    so a single ACTIVATE computes both heads' exp; fully-masked columns
    of straddle k-tiles are skipped in the logits/AV matmuls and only
    the diagonal 128x128 block is masked,
  - QKV / output-projection accumulation chains are interleaved between
    attention groups as PE filler so the tensor engine never micro-idles
    (keeps the HAM clock gate at 8/8),
  - per (q-chunk, pair): AllGather of the pair's normalized ctx^T within
    the 4-core batch group; the output projection accumulates all 8
    gathered c-tiles in PSUM and writes a disjoint 256-column slice.
Host-side: shard prep (pack tensors into SBUF-layout DRAM images) and a
concat of the 8 output column-slices.  All FLOPs on device.
"""

import numpy as np
import ml_dtypes

import concourse.bass as bass
import concourse.mybir as mybir
import concourse.tile as tile
from concourse import bacc
from concourse import bass_utils

BF16 = mybir.dt.bfloat16
F32 = mybir.dt.float32
AF = mybir.ActivationFunctionType

T = 2048
D = 1024
NH = 16
HPC = 4  # heads per core
DK = 64
NCORES = 8
TQ = 512  # q-chunk
NQC = T // TQ  # 4
NDT = D // 128  # 8 d-tiles
VW = DK + 1  # 65: v columns per head incl. ones column
WV = HPC * VW  # 260
SCALE = 1.0 / 8.0  # 1/sqrt(DK)

TRACE = False
LAST_RESULT = {}

_cache = {}


def _build():
    nc = bacc.Bacc("TRN2", target_bir_lowering=False, debug=False,
                   num_devices=NCORES)

    xt = nc.declare_dram_parameter("xt", [128, NQC * NDT * TQ], BF16, False)
    wq = nc.declare_dram_parameter("wq", [128, NDT * 256], BF16, False)
    wk = nc.declare_dram_parameter("wk", [128, NDT * 256], BF16, False)
    wv = nc.declare_dram_parameter("wv", [128, NDT * WV], BF16, False)
    wo = nc.declare_dram_parameter("wo", [128, NDT * 256], BF16, False)
    bq = nc.declare_dram_parameter("bq", [128, 2], F32, False)
    bk = nc.declare_dram_parameter("bk", [128, 2], F32, False)
    bv = nc.declare_dram_parameter("bv", [128, WV], F32, False)
    bo = nc.declare_dram_parameter("bo", [128, 256], F32, False)
    mask = nc.declare_dram_parameter("mask", [128, 128], BF16, False)
    out = nc.declare_dram_parameter("out", [T, 256], BF16, True)

    # one AllGather per (q-chunk, head-pair): 2 heads x 64 rows in,
    # 4 cores x 128 rows out
    cc_in = [[nc.dram_tensor(f"cc_in{qc}_{p}", [2 * DK, TQ], BF16)
              for p in range(2)] for qc in range(NQC)]
    cc_out = [[nc.dram_tensor(f"cc_out{qc}_{p}", [8 * DK, TQ], BF16)
               for p in range(2)] for qc in range(NQC)]

    with tile.TileContext(nc) as tc, tc.tile_pool(name="pers", bufs=1) as pers:
        # ---------------- persistent SBUF ----------------
        xt_sb = pers.tile([128, NQC * NDT * TQ], BF16, tag="xt_sb", name="xt_sb")
        wq_sb = pers.tile([128, NDT * 256], BF16, tag="wq_sb", name="wq_sb")
        wk_sb = pers.tile([128, NDT * 256], BF16, tag="wk_sb", name="wk_sb")
        wv_sb = pers.tile([128, NDT * WV], BF16, tag="wv_sb", name="wv_sb")
        wo_sb = pers.tile([128, NDT * 256], BF16, tag="wo_sb", name="wo_sb")
        bq_sb = pers.tile([128, 2], F32, tag="bq_sb", name="bq_sb")
        bk_sb = pers.tile([128, 2], F32, tag="bk_sb", name="bk_sb")
        bv_sb = pers.tile([128, WV], F32, tag="bv_sb", name="bv_sb")
        bo_sb = pers.tile([128, 256], F32, tag="bo_sb", name="bo_sb")
        mask_sb = pers.tile([128, 128], BF16, tag="mask_sb", name="mask_sb")
        qT_sb = pers.tile([128, 2 * T], BF16, tag="qT_sb", name="qT_sb")
        kT_sb = pers.tile([128, 2 * T], BF16, tag="kT_sb", name="kT_sb")
        v_sb = pers.tile([128, (T // 128) * WV], BF16, tag="v_sb", name="v_sb")
        ctxg_sb = pers.tile([128, NQC * 8 * TQ], BF16, tag="ctxg_sb",
                            name="ctxg_sb")

        # input DMAs in first-need order (sync ring is in-order; all rings
        # share the per-core HBM bandwidth, so one ordered ring is best)
        nc.sync.dma_start(wq_sb[:, 0:1024], wq[:, 0:1024])
        nc.sync.dma_start(xt_sb[:, 0:NDT * TQ // 2], xt[:, 0:NDT * TQ // 2])
        nc.sync.dma_start(xt_sb[:, NDT * TQ // 2:NDT * TQ],
                          xt[:, NDT * TQ // 2:NDT * TQ])
        nc.sync.dma_start(wk_sb[:, 0:1024], wk[:, 0:1024])
        nc.sync.dma_start(wq_sb[:, 1024:2048], wq[:, 1024:2048])
        nc.sync.dma_start(wk_sb[:, 1024:2048], wk[:, 1024:2048])
        nc.sync.dma_start(wv_sb[:], wv[:])
        nc.sync.dma_start(bq_sb[:], bq[:])
        nc.sync.dma_start(bk_sb[:], bk[:])
        nc.sync.dma_start(bv_sb[:], bv[:])
        nc.sync.dma_start(mask_sb[:], mask[:])
        nc.sync.dma_start(xt_sb[:, NDT * TQ:2 * NDT * TQ],
                          xt[:, NDT * TQ:2 * NDT * TQ])
        nc.sync.dma_start(xt_sb[:, 2 * NDT * TQ:3 * NDT * TQ],
                          xt[:, 2 * NDT * TQ:3 * NDT * TQ])
        nc.sync.dma_start(xt_sb[:, 3 * NDT * TQ:4 * NDT * TQ],
                          xt[:, 3 * NDT * TQ:4 * NDT * TQ])
        nc.sync.dma_start(wo_sb[:], wo[:])
        nc.sync.dma_start(bo_sb[:], bo[:])

        with (
            tc.tile_pool(name="pp", space="PSUM", bufs=1) as pp,
            tc.tile_pool(name="sp", space="SBUF", bufs=2) as sp,
        ):
            # PSUM budget (8 banks): lg [128,2048]f32 = 4, ctxX/ctxY
            # [65,512]f32 = 1+1, po (QKV/proj filler ring) [128,512] x2 = 2.

            # ---- filler chains: QKV and output-projection accumulation
            # chains, emitted one per attention group to keep PE dense ----
            def mk_qk(which, p, qc):
                w_sb, b_sb, o_sb = ((wq_sb, bq_sb, qT_sb) if which == "q"
                                    else (wk_sb, bk_sb, kT_sb))

                def emit():
                    x0 = qc * NDT * TQ
                    ps = pp.tile([128, TQ], F32, tag="po", bufs=2,
                                 name=f"ps{which}_{p}_{qc}")
                    for d in range(NDT):
                        nc.tensor.matmul(
                            ps[:],
                            lhsT=w_sb[:, p * 1024 + d * 128:
                                      p * 1024 + d * 128 + 128],
                            rhs=xt_sb[:, x0 + d * TQ:x0 + (d + 1) * TQ],
                            start=(d == 0), stop=(d == NDT - 1))
                    nc.vector.tensor_scalar_add(
                        o_sb[:, p * T + qc * TQ:p * T + (qc + 1) * TQ],
                        ps[:], b_sb[:, p:p + 1])
                return emit

            def mk_v(tt):
                def emit():
                    qc, tl = tt // 4, tt % 4
                    x0 = qc * NDT * TQ
                    psv = pp.tile([128, WV], F32, tag="po", bufs=2,
                                  name=f"psv_{tt}")
                    for d in range(NDT):
                        nc.tensor.matmul(
                            psv[:],
                            lhsT=xt_sb[:, x0 + d * TQ + tl * 128:
                                       x0 + d * TQ + (tl + 1) * 128],
                            rhs=wv_sb[:, d * WV:(d + 1) * WV],
                            start=(d == 0), stop=(d == NDT - 1))
                    nc.vector.tensor_add(v_sb[:, tt * WV:(tt + 1) * WV],
                                         psv[:], bv_sb[:])
                return emit

            out_tiles = {}

            def emit_outs(qc):
                # grouped store DMAs: emitted at points where they cannot
                # delay a later ctxg transfer on the in-order sync ring
                for tl in range(4):
                    tt = 4 * qc + tl
                    nc.sync.dma_start(out[128 * tt:128 * (tt + 1), :],
                                      out_tiles[tt][:])

            def mk_proj_part(qc, tl, part, pod):
                # part 0: c-tiles 0-3 (opens the PSUM accumulation);
                # part 1: c-tiles 4-7 (closes it) + bias + store.
                def emit():
                    tt = 4 * qc + tl
                    if part == 0:
                        pod[tl] = pp.tile([128, 256], F32, tag="po", bufs=2,
                                          name=f"po_{tt}")
                    po = pod[tl]
                    for c in (range(4) if part == 0 else range(4, 8)):
                        nc.tensor.matmul(
                            po[:],
                            lhsT=ctxg_sb[:, (8 * qc + c) * TQ + tl * 128:
                                         (8 * qc + c) * TQ + (tl + 1) * 128],
                            rhs=wo_sb[:, c * 256:(c + 1) * 256],
                            start=(c == 0), stop=(c == 7))
                    if part == 1:
                        o_sb = sp.tile([128, 256], BF16, tag="o_sb", bufs=16,
                                       name=f"o_{tt}")
                        nc.vector.tensor_add(o_sb[:], po[:], bo_sb[:])
                        out_tiles[tt] = o_sb
                return emit

            def mk_proj(qc, tl):
                pod = {}
                a, b = mk_proj_part(qc, tl, 0, pod), mk_proj_part(qc, tl, 1, pod)

                def emit():
                    a()
                    b()
                return emit

            def proj_halves(qc):
                pod = {}
                return [mk_proj_part(qc, tl, part, pod)
                        for tl in range(4) for part in (0, 1)]

            def qkv_chains(qc):
                return [mk_qk("q", 0, qc), mk_qk("k", 0, qc),
                        mk_qk("q", 1, qc), mk_qk("k", 1, qc),
                        mk_v(4 * qc + 0), mk_v(4 * qc + 1),
                        mk_v(4 * qc + 2), mk_v(4 * qc + 3)]

            # ---------------- attention ----------------
            # Pair p covers local heads (2p, 2p+1): chain X at partition
            # rows 0-63, chain Y at rows 64-127 of the qT/kT p-block.
            # Both chains' logits land in one [128, 2048] PSUM tile ->
            # single exp ACTIVATE per 2-k-tile group.  AV matmuls are
            # software-pipelined one group behind the logits matmuls so
            # the next group's logits can issue during this group's exp.
            def emit_attn(p, qc, fillers):
                nkt = 4 * qc + 4
                chains = [(2 * p, 0, 0, "X"), (2 * p + 1, 64, 2 * TQ, "Y")]
                ctxs = {}
                pend = [None]

                def make_av(grp, ex):
                    def emit():
                        for h, r0, base, cn in chains:
                            for j in range(2):
                                kt = 2 * grp + j
                                r = kt - 4 * qc
                                c0 = 128 * r if r >= 0 else 0
                                nc.tensor.matmul(
                                    ctxs[cn][:, c0:TQ],
                                    lhsT=v_sb[:, kt * WV + VW * h:
                                              kt * WV + VW * h + VW],
                                    rhs=ex[:, base + j * TQ + c0:
                                           base + (j + 1) * TQ],
                                    start=(kt == 0), stop=(kt == nkt - 1))
                    return emit

                for grp in range(nkt // 2):
                    if fillers:
                        fillers.pop(0)()
                    lg = pp.tile([128, 4 * TQ], F32, tag="lg", bufs=1,
                                 name=f"lg_{p}_{qc}_{grp}")
                    ex = sp.tile([128, 4 * TQ], BF16, tag="ex", bufs=4,
                                 name=f"ex_{p}_{qc}_{grp}")
                    if grp == 0:
                        for h, r0, base, cn in chains:
                            ctxs[cn] = pp.tile([VW, TQ], F32, tag=f"ctx{cn}",
                                               bufs=1, name=f"ctx_{h}_{qc}")
                    for j in range(2):
                        kt = 2 * grp + j
                        r = kt - 4 * qc
                        c0 = 128 * r if r >= 0 else 0
                        for h, r0, base, cn in chains:
                            nc.tensor.matmul(
                                lg[:, base + j * TQ + c0:base + (j + 1) * TQ],
                                lhsT=kT_sb[r0:r0 + DK,
                                           p * T + kt * 128:p * T + (kt + 1) * 128],
                                rhs=qT_sb[r0:r0 + DK,
                                          p * T + qc * TQ + c0:
                                          p * T + (qc + 1) * TQ],
                                start=True, stop=True)
                    # previous group's AV matmuls go after this group's
                    # logits in the tensor queue (they run during exp)
                    if pend[0] is not None:
                        pend[0]()
                    # second straddle group's valid cols all lie >= 256
                    e0 = 256 if grp == 2 * qc + 1 else 0
                    nc.scalar.activation(ex[:, e0:4 * TQ], lg[:, e0:4 * TQ],
                                         AF.Exp, scale=SCALE)
                    for h, r0, base, cn in chains:
                        for j in range(2):
                            kt = 2 * grp + j
                            r = kt - 4 * qc
                            if r >= 0:
                                nc.vector.tensor_mul(
                                    ex[:, base + j * TQ + 128 * r:
                                       base + j * TQ + 128 * (r + 1)],
                                    ex[:, base + j * TQ + 128 * r:
                                       base + j * TQ + 128 * (r + 1)],
                                    mask_sb[:])
                    pend[0] = make_av(grp, ex)
                pend[0]()
                # normalize + ship to the collective input buffer
                for h, r0, base, cn in chains:
                    ctxf = sp.tile([VW, TQ], F32, tag=f"cf{cn}", bufs=2,
                                   name=f"cf_{h}_{qc}")
                    nc.vector.tensor_copy(ctxf[:], ctxs[cn][:])
                    dn = sp.tile([1, TQ], F32, tag=f"dn{cn}", bufs=2,
                                 name=f"dn_{h}_{qc}")
                    nc.vector.tensor_copy(dn[:], ctxf[DK:DK + 1, :])
                    rc = sp.tile([1, TQ], F32, tag=f"rc{cn}", bufs=2,
                                 name=f"rc_{h}_{qc}")
                    nc.vector.reciprocal_approx_fast(rc[:], dn[:])
                    rcb = sp.tile([DK, TQ], F32, tag=f"rcb{cn}", bufs=2,
                                  name=f"rcb_{h}_{qc}")
                    nc.gpsimd.partition_broadcast(rcb[:], rc[:])
                    ctxd = sp.tile([DK, TQ], BF16, tag=f"cd{cn}", bufs=2,
                                   name=f"cd_{h}_{qc}")
                    nc.vector.tensor_mul(ctxd[:], ctxf[0:DK, :], rcb[:])
                    nc.gpsimd.dma_start(
                        cc_in[qc][p][DK * (h - 2 * p):DK * (h - 2 * p + 1), :],
                        ctxd[:])

            def emit_ag(qc, p):
                nc.gpsimd.collective_compute(
                    "AllGather",
                    mybir.AluOpType.bypass,
                    replica_groups=[[0, 1, 2, 3], [4, 5, 6, 7]],
                    ins=[cc_in[qc][p].ap().opt()],
                    outs=[cc_out[qc][p].ap().opt()],
                )
                # c-tile slots: AG-A (pair 0) -> c 0..3, AG-B -> c 4..7
                for c in range(4):
                    nc.sync.dma_start(
                        ctxg_sb[:, (8 * qc + 4 * p + c) * TQ:
                                (8 * qc + 4 * p + c + 1) * TQ],
                        cc_out[qc][p][128 * c:128 * (c + 1), :])

            # ---------------- schedule ----------------
            for f in qkv_chains(0):
                f()
            qkv1 = qkv_chains(1)
            emit_attn(0, 0, qkv1)          # 2 groups -> 2 fillers
            emit_ag(0, 0)
            emit_attn(1, 0, qkv1)          # 2 more
            emit_ag(0, 1)
            for f in qkv1:                 # 4 leftover qc1 chains
                f()
            qkv2 = qkv_chains(2)
            emit_attn(0, 1, qkv2)          # 4 fillers
            emit_ag(1, 0)
            emit_attn(1, 1, qkv2)          # 4 fillers
            emit_ag(1, 1)
            qkv3 = qkv_chains(3)
            p0h = proj_halves(0)
            p1h = proj_halves(1)
            p2h = proj_halves(2)
            p3h = proj_halves(3)
            emit_attn(0, 2, qkv3[0:4] + p0h[0:2])  # 6 groups, 6 fillers
            emit_ag(2, 0)
            emit_attn(1, 2, qkv3[4:8] + p0h[2:4])  # 6 groups, 6 fillers
            emit_ag(2, 1)
            emit_attn(0, 3, p0h[4:8] + p1h[0:4])   # 8 groups, 8 fillers
            emit_ag(3, 0)
            emit_outs(0)
            emit_attn(1, 3, p1h[4:8])  # light fillers: finish divisions early
            emit_ag(3, 1)
            emit_outs(1)
            for f in p2h:                  # overlaps the last AllGather
                f()
            emit_outs(2)
            # proj(3): A-parts (from AG-A qc3) run during the AG-B wait;
            # B-parts are the true tail
            for i in (0, 2, 1, 3, 4, 6, 5, 7):
                p3h[i]()
            emit_outs(3)

    nc.compile()
    return nc


def _pack_rows_pmajor(a):
    """[1024, 256] -> [128, p*1024 + d*128 + n] (p = 128-col block of a)."""
    r = a.reshape(NDT, 128, 2, 128).transpose(1, 2, 0, 3)
    return np.ascontiguousarray(r.reshape(128, 2048))


def _pack_rows(a):
    """[1024, N] -> [128, 8*N] with block d at cols [d*N, (d+1)*N)."""
    n = a.shape[1]
    return np.ascontiguousarray(
        a.reshape(NDT, 128, n).transpose(1, 0, 2).reshape(128, NDT * n))


def _shard_inputs(x, Wqkv, bqkv, Wo, bo_v):
    bf = ml_dtypes.bfloat16
    jj = np.arange(128)[:, None]
    ii = np.arange(128)[None, :]
    mask = (jj <= ii).astype(np.float32).astype(bf)
    in_maps = []
    for c in range(NCORES):
        b, g = c // 4, c % 4
        h0 = 4 * g
        q0 = h0 * DK
        # x[b] [T, D] -> [128, qc*4096 + d*512 + i]
        xr = x[b].reshape(NQC, TQ, NDT, 128).transpose(3, 0, 2, 1)
        xr = np.ascontiguousarray(xr.reshape(128, NQC * NDT * TQ)).astype(bf)
        wv_full = np.zeros((D, WV), np.float32)
        bv_full = np.zeros((WV,), np.float32)
        for a in range(HPC):
            wv_full[:, VW * a:VW * a + DK] = Wqkv[:, 2 * D + (h0 + a) * DK:
                                                  2 * D + (h0 + a + 1) * DK]
            bv_full[VW * a:VW * a + DK] = bqkv[2 * D + (h0 + a) * DK:
                                               2 * D + (h0 + a + 1) * DK]
            bv_full[VW * a + DK] = 1.0
        # W_o rows in c-tile order: c<4 from AG-A (pair 0), c>=4 from
        # AG-B (pair 1); tile c holds heads 4*(c%4) + pair_off + {0,1}
        wo_blocks = []
        for ct in range(8):
            j = ct % 4
            off = 0 if ct < 4 else 2
            for half in range(2):
                h = 4 * j + off + half
                wo_blocks.append(Wo[h * DK:(h + 1) * DK, 256 * g:256 * (g + 1)])
        wo_r = np.concatenate(wo_blocks, axis=0)
        in_maps.append({
            "xt": xr,
            "wq": _pack_rows_pmajor(Wqkv[:, q0:q0 + 256]).astype(bf),
            "wk": _pack_rows_pmajor(Wqkv[:, D + q0:D + q0 + 256]).astype(bf),
            "wv": _pack_rows(wv_full).astype(bf),
            "wo": _pack_rows(wo_r).astype(bf),
            "bq": np.stack([bqkv[q0:q0 + 128], bqkv[q0 + 128:q0 + 256]],
                           axis=1).astype(np.float32).copy(),
            "bk": np.stack([bqkv[D + q0:D + q0 + 128],
                            bqkv[D + q0 + 128:D + q0 + 256]],
                           axis=1).astype(np.float32).copy(),
            "bv": np.ascontiguousarray(
                np.broadcast_to(bv_full, (128, WV))).astype(np.float32),
            "bo": np.ascontiguousarray(
                np.broadcast_to(bo_v[256 * g:256 * (g + 1)], (128, 256))
            ).astype(np.float32),
            "mask": mask,
        })
    return in_maps


def kernel(**inputs):
    x = np.asarray(inputs["x"], np.float32)
    Wqkv = np.asarray(inputs["W_qkv"], np.float32)
    bqkv = np.asarray(inputs["b_qkv"], np.float32)
    Wo = np.asarray(inputs["W_o"], np.float32)
    bo_v = np.asarray(inputs["b_o"], np.float32)

    if "nc" not in _cache:
        _cache["nc"] = _build()
    nc = _cache["nc"]

    in_maps = _shard_inputs(x, Wqkv, bqkv, Wo, bo_v)
    res = bass_utils.run_bass_kernel_spmd(
        nc, in_maps, core_ids=list(range(NCORES)), trace=TRACE)
    LAST_RESULT["exec_time_ns"] = res.exec_time_ns
    LAST_RESULT["res"] = res

    out = np.empty((2, T, D), np.float32)
    for c in range(NCORES):
        out[c // 4, :, 256 * (c % 4):256 * (c % 4 + 1)] = \
            np.asarray(res.results[c]["out"], np.float32)
    return out
